# revision 14
# baseline (speedup 1.0000x reference)
"""Minkowski attention TRN2 kernel (8 NeuronCores, SPMD).

Sharding: batch x query-quarter (T-sharding). Core c handles b = c//4 and
query rows qs = (c%4)*512 .. +512, all 16 heads. No collectives needed:
each core produces attn[b, :, qs:qs+512, :] and out[b, qs:qs+512, :].

Math per core:
  scores = (q @ k^T) * scale - relu(ds2)
  attn   = softmax(scores) = exp(qk*scale) * m / rowsum,  m = min(1, exp(-ds2))
  ds2 is computed on PE as a rank-5 outer product:  -ds2[i,j] = g_i . f_j
  with g = [-u, -1, -2t, 2a1, 2a2], f = [1, u, t, a1, a2], u = -t^2+a1^2+a2^2.

Both score layouts are computed on PE (no transposes):
  S   [q,k]  -> exp -> *m -> rowsum/normalize -> attn output (HBM, cast to f32)
  S^T [k,q]  -> exp -> *m -> PV matmul with V_aug (ones col gives colsums)
out^T = V_aug^T @ pm^T, normalized per-q via a broadcast multiply, then the
output projection runs locally (lhsT = attn_out^T), with bias folded as
bo_eff = bo + bv @ Wo (since rows of attn sum to 1).
"""

import numpy as np

import concourse.bacc as bacc
import concourse.bass as bass
import concourse.mybir as mybir
import concourse.tile as tile
from concourse import bass_utils

B, T, D, H, HD = 2, 2048, 1024, 16, 64
SCALE = HD ** -0.5
NCORES = 8
QS = T // 4  # 512 query rows per core

F32 = mybir.dt.float32
BF16 = mybir.dt.bfloat16
AF = mybir.ActivationFunctionType
ALU = mybir.AluOpType

_CACHE = {}


def _build_program():
    nc = bacc.Bacc("TRN2", target_bir_lowering=False, debug=False)

    # ---- DRAM I/O (same program on every core; per-core data differs) ----
    xT = nc.dram_tensor("xT", [D, T], F32, kind="ExternalInput").ap()
    xTq = nc.dram_tensor("xTq", [D, QS], F32, kind="ExternalInput").ap()
    Wq = nc.dram_tensor("Wq", [D, D], F32, kind="ExternalInput").ap()
    Wk = nc.dram_tensor("Wk", [D, D], F32, kind="ExternalInput").ap()
    Wv = nc.dram_tensor("Wv", [D, D], F32, kind="ExternalInput").ap()
    Wo = nc.dram_tensor("Wo", [D, D], F32, kind="ExternalInput").ap()
    bqr = nc.dram_tensor("bqr", [128, 8], F32, kind="ExternalInput").ap()
    bkr = nc.dram_tensor("bkr", [128, 8], F32, kind="ExternalInput").ap()
    bvr = nc.dram_tensor("bvr", [128, 8], F32, kind="ExternalInput").ap()
    bo = nc.dram_tensor("bo", [1, D], F32, kind="ExternalInput").ap()
    G = nc.dram_tensor("G", [5, T], F32, kind="ExternalInput").ap()
    F = nc.dram_tensor("Fm", [5, T], F32, kind="ExternalInput").ap()
    Gq = nc.dram_tensor("Gq", [5, QS], F32, kind="ExternalInput").ap()
    Fq = nc.dram_tensor("Fq", [5, QS], F32, kind="ExternalInput").ap()
    attn_out = nc.dram_tensor("attn_part", [H, QS, T], F32, kind="ExternalOutput").ap()
    out_part = nc.dram_tensor("out_part", [QS, D], F32, kind="ExternalOutput").ap()

    with tile.TileContext(nc) as tc:
        with tc.tile_pool(name="consts", bufs=1) as consts:
            # small persistents
            bq_sb = consts.tile([128, 8], F32)
            bk_sb = consts.tile([128, 8], F32)
            bv_sb = consts.tile([128, 8], F32)
            ones_sb = consts.tile([1, 128], F32)
            for dst, src in ((bq_sb, bqr), (bk_sb, bkr), (bv_sb, bvr)):
                nc.sync.dma_start(out=dst, in_=src)
            nc.vector.memset(ones_sb, 1.0)

            # big persistents  (~106 KB/partition)
            QT_sb = consts.tile([128, 8, QS], BF16)       # q^T  [do, q]
            KT_sb = consts.tile([128, 8, T], BF16)        # k^T  [do, tok]
            V_sb = consts.tile([128, 16, H, 65], BF16)    # v    [tok, h, hd+1]
            m1_sb = consts.tile([128, 4, T], BF16)        # m[q, k]
            m2_sb = consts.tile([128, 16, QS], BF16)      # m[k, q]

            # ---------------- m precompute ----------------
            with tc.tile_pool(name="feat", bufs=1) as fp, \
                 tc.tile_pool(name="mpsum", bufs=4, space="PSUM") as mp:
                G_sb = fp.tile([5, T], F32)
                F_sb = fp.tile([5, T], F32)
                Gq_sb = fp.tile([5, QS], F32)
                Fq_sb = fp.tile([5, QS], F32)
                for dst, src in ((G_sb, G), (F_sb, F), (Gq_sb, Gq), (Fq_sb, Fq)):
                    nc.sync.dma_start(out=dst, in_=src)
                for qt in range(4):
                    for kc in range(4):
                        ps = mp.tile([128, 512], F32)
                        nc.tensor.matmul(
                            ps, lhsT=Gq_sb[:, qt * 128:(qt + 1) * 128],
                            rhs=F_sb[:, kc * 512:(kc + 1) * 512],
                            start=True, stop=True)
                        dst = m1_sb[:, qt, kc * 512:(kc + 1) * 512]
                        nc.scalar.activation(out=dst, in_=ps, func=AF.Exp)
                        nc.vector.tensor_scalar_min(dst, dst, 1.0)
                for kt in range(16):
                    ps = mp.tile([128, 512], F32)
                    nc.tensor.matmul(
                        ps, lhsT=G_sb[:, kt * 128:(kt + 1) * 128], rhs=Fq_sb,
                        start=True, stop=True)
                    dst = m2_sb[:, kt, :]
                    nc.scalar.activation(out=dst, in_=ps, func=AF.Exp)
                    nc.vector.tensor_scalar_min(dst, dst, 1.0)

            # ---------------- projections ----------------
            # phase 1: Q^T   (lhsT = Wq chunk, rhs = x^T chunk)
            with tc.tile_pool(name="wqp", bufs=1) as wpool, \
                 tc.tile_pool(name="xts", bufs=1) as xts, \
                 tc.tile_pool(name="pp", bufs=4, space="PSUM") as pp:
                wq_sb = wpool.tile([128, 8, D], F32)
                nc.sync.dma_start(out=wq_sb, in_=Wq.rearrange("(c p) d -> p c d", p=128))
                xtq_tiles = []
                for di in range(8):
                    t_ = xts.tile([128, 512], F32, name=f"xtq{di}", tag=f"xtq{di}")
                    nc.sync.dma_start(out=t_, in_=xTq[di * 128:(di + 1) * 128, :])
                    xtq_tiles.append(t_)
                for do_t in range(8):
                    ps = pp.tile([128, 512], F32)
                    for di in range(8):
                        nc.tensor.matmul(
                            ps, lhsT=wq_sb[:, di, do_t * 128:(do_t + 1) * 128],
                            rhs=xtq_tiles[di], start=(di == 0), stop=(di == 7))
                    nc.scalar.activation(
                        out=QT_sb[:, do_t, :], in_=ps, func=AF.Identity,
                        bias=bq_sb[:, do_t:do_t + 1])

            # phase 2: K^T
            with tc.tile_pool(name="wkp", bufs=1) as wpool, \
                 tc.tile_pool(name="xts2", bufs=10) as xts, \
                 tc.tile_pool(name="pp2", bufs=4, space="PSUM") as pp:
                wk_sb = wpool.tile([128, 8, D], F32)
                nc.sync.dma_start(out=wk_sb, in_=Wk.rearrange("(c p) d -> p c d", p=128))
                for tc_i in range(4):
                    xt_tiles = []
                    for di in range(8):
                        t_ = xts.tile([128, 512], F32, name="xt", tag="xt")
                        nc.sync.dma_start(
                            out=t_,
                            in_=xT[di * 128:(di + 1) * 128, tc_i * 512:(tc_i + 1) * 512])
                        xt_tiles.append(t_)
                    for do_t in range(8):
                        ps = pp.tile([128, 512], F32)
                        for di in range(8):
                            nc.tensor.matmul(
                                ps, lhsT=wk_sb[:, di, do_t * 128:(do_t + 1) * 128],
                                rhs=xt_tiles[di], start=(di == 0), stop=(di == 7))
                        nc.scalar.activation(
                            out=KT_sb[:, do_t, tc_i * 512:(tc_i + 1) * 512], in_=ps,
                            func=AF.Identity, bias=bk_sb[:, do_t:do_t + 1])

            # phase 3: V (natural layout; lhsT = x^T chunk, rhs = Wv chunk)
            with tc.tile_pool(name="wvp", bufs=1) as wpool, \
                 tc.tile_pool(name="xts3", bufs=10) as xts, \
                 tc.tile_pool(name="pp3", bufs=4, space="PSUM") as pp:
                wv_sb = wpool.tile([128, 8, D], F32)
                nc.sync.dma_start(out=wv_sb, in_=Wv.rearrange("(c p) d -> p c d", p=128))
                for tc_i in range(4):
                    xt_tiles = []
                    for di in range(8):
                        t_ = xts.tile([128, 512], F32, name="xt2", tag="xt2")
                        nc.sync.dma_start(
                            out=t_,
                            in_=xT[di * 128:(di + 1) * 128, tc_i * 512:(tc_i + 1) * 512])
                        xt_tiles.append(t_)
                    for tt in range(4):
                        c = tc_i * 4 + tt
                        for dc in range(2):
                            ps = pp.tile([128, 512], F32)
                            for di in range(8):
                                nc.tensor.matmul(
                                    ps,
                                    lhsT=xt_tiles[di][:, tt * 128:(tt + 1) * 128],
                                    rhs=wv_sb[:, di, dc * 512:(dc + 1) * 512],
                                    start=(di == 0), stop=(di == 7))
                            nc.vector.tensor_copy(
                                out=V_sb[:, c, dc * 8:(dc + 1) * 8, 0:64],
                                in_=ps.rearrange("p (h e) -> p h e", e=64))
                nc.vector.memset(V_sb[:, :, :, 64:65], 1.0)

            # ---------------- main attention loop ----------------
            with tc.tile_pool(name="wop", bufs=1) as wopool, \
                 tc.tile_pool(name="pmt", bufs=2) as pmtp, \
                 tc.tile_pool(name="pm", bufs=3) as pmp, \
                 tc.tile_pool(name="small", bufs=4) as sp, \
                 tc.tile_pool(name="bc", bufs=2) as bcp:
                wo_sb = wopool.tile([128, 8, D], F32)
                nc.sync.dma_start(out=wo_sb, in_=Wo.rearrange("(c p) d -> p c d", p=128))
                bo_sb = wopool.tile([1, D], F32)
                boeff_sb = wopool.tile([1, D], F32)
                AOT_sb = wopool.tile([128, 8, QS], F32)   # attn_out^T [hd, q]
                nc.sync.dma_start(out=bo_sb, in_=bo)

                # bo_eff = bo + bv @ Wo
                with tc.tile_pool(name="bops", bufs=2, space="PSUM") as bop:
                    for dc in range(2):
                        bps = bop.tile([1, 512], F32)
                        for di in range(8):
                            nc.tensor.matmul(
                                bps, lhsT=bv_sb[:, di:di + 1],
                                rhs=wo_sb[:, di, dc * 512:(dc + 1) * 512],
                                start=(di == 0), stop=False)
                        nc.tensor.matmul(
                            bps, lhsT=ones_sb[0:1, 0:1],
                            rhs=bo_sb[0:1, dc * 512:(dc + 1) * 512],
                            start=False, stop=True)
                        nc.vector.tensor_copy(
                            out=boeff_sb[0:1, dc * 512:(dc + 1) * 512], in_=bps)

                with tc.tile_pool(name="ps", bufs=2, space="PSUM") as psp, \
                     tc.tile_pool(name="upool", bufs=2, space="PSUM") as up, \
                     tc.tile_pool(name="trp", bufs=2, space="PSUM") as trp:
                    recip_cols = {}

                    def emit_a_c(g):
                        """S^T side for group g: pm^T, PV accumulation, and
                        colsum-normalized attn_out^T."""
                        heads = (2 * g, 2 * g + 1)
                        u_tiles = {}
                        for h in heads:
                            u_tiles[h] = up.tile([128, 512], F32,
                                                 name=f"u_{h}", tag="u")
                        for c in range(16):
                            pst = psp.tile([128, 1024], F32, name="pst", tag="ps")
                            pmt = pmtp.tile([128, 1024], BF16)
                            for j, h in enumerate(heads):
                                po = (h % 2) * 64
                                nc.tensor.matmul(
                                    pst[:, j * 512:(j + 1) * 512],
                                    lhsT=KT_sb[po:po + 64, h // 2,
                                               c * 128:(c + 1) * 128],
                                    rhs=QT_sb[po:po + 64, h // 2, :],
                                    start=True, stop=True)
                            nc.scalar.activation(out=pmt, in_=pst, func=AF.Exp,
                                                 scale=SCALE)
                            for j, h in enumerate(heads):
                                sl = pmt[:, j * 512:(j + 1) * 512]
                                nc.vector.tensor_mul(sl, sl, m2_sb[:, c, :])
                                nc.tensor.matmul(
                                    u_tiles[h][0:65, :],
                                    lhsT=V_sb[:, c, h, :],
                                    rhs=sl,
                                    start=(c == 0), stop=(c == 15))
                        for h in heads:
                            po = (h % 2) * 64
                            recipT = sp.tile([1, 512], F32, name="recipT",
                                             tag="recipT", bufs=4)
                            nc.vector.reciprocal(recipT, u_tiles[h][64:65, :])
                            bcast = bcp.tile([64, 512], F32, name="bcast", tag="bc")
                            nc.gpsimd.partition_broadcast(bcast, recipT)
                            nc.vector.tensor_mul(
                                AOT_sb[po:po + 64, h // 2, :],
                                u_tiles[h][0:64, :], bcast)
                            # transpose recipT -> per-q-tile columns (K=1 matmul)
                            rc = trp.tile([128, 4], F32, name="rc", tag="rc")
                            for qt in range(4):
                                nc.tensor.matmul(
                                    rc[:, qt:qt + 1],
                                    lhsT=recipT[0:1, qt * 128:(qt + 1) * 128],
                                    rhs=ones_sb[0:1, 0:1],
                                    start=True, stop=True)
                            rcs = sp.tile([128, 4], F32, name="rcs", tag="rcs",
                                          bufs=4)
                            nc.vector.tensor_copy(out=rcs, in_=rc)
                            recip_cols[h] = rcs

                    def emit_b(g):
                        """S side for group g: normalized attn rows to HBM."""
                        for h in (2 * g, 2 * g + 1):
                            po = (h % 2) * 64
                            for qt in range(4):
                                pm = pmp.tile([128, T], BF16, name="pm", tag="pm")
                                for half in range(2):
                                    psb = psp.tile([128, 1024], F32,
                                                   name="psb", tag="ps")
                                    for kc in range(2):
                                        ko = (half * 2 + kc) * 512
                                        nc.tensor.matmul(
                                            psb[:, kc * 512:(kc + 1) * 512],
                                            lhsT=QT_sb[po:po + 64, h // 2,
                                                       qt * 128:(qt + 1) * 128],
                                            rhs=KT_sb[po:po + 64, h // 2,
                                                      ko:ko + 512],
                                            start=True, stop=True)
                                    sl = pm[:, half * 1024:(half + 1) * 1024]
                                    nc.scalar.activation(out=sl, in_=psb,
                                                         func=AF.Exp, scale=SCALE)
                                    nc.vector.tensor_mul(
                                        sl, sl,
                                        m1_sb[:, qt, half * 1024:(half + 1) * 1024])
                                nc.vector.tensor_scalar_mul(
                                    pm, pm, recip_cols[h][:, qt:qt + 1])
                                nc.gpsimd.dma_start(
                                    out=attn_out[h, qt * 128:(qt + 1) * 128, :],
                                    in_=pm)

                    # software pipeline: overlap group g's S^T phase with
                    # group g-1's S phase
                    emit_a_c(0)
                    for g in range(1, 8):
                        emit_a_c(g)
                        emit_b(g - 1)
                    emit_b(7)

                # ---------------- output projection ----------------
                with tc.tile_pool(name="ops", bufs=2, space="PSUM") as opp, \
                     tc.tile_pool(name="osb", bufs=2) as osb:
                    for qt in range(4):
                        for dc in range(2):
                            ops = opp.tile([128, 512], F32)
                            for hd_t in range(8):
                                nc.tensor.matmul(
                                    ops,
                                    lhsT=AOT_sb[:, hd_t, qt * 128:(qt + 1) * 128],
                                    rhs=wo_sb[:, hd_t, dc * 512:(dc + 1) * 512],
                                    start=(hd_t == 0), stop=False)
                            nc.tensor.matmul(
                                ops, lhsT=ones_sb[0:1, :],
                                rhs=boeff_sb[0:1, dc * 512:(dc + 1) * 512],
                                start=False, stop=True)
                            ot = osb.tile([128, 512], F32)
                            nc.vector.tensor_copy(out=ot, in_=ops)
                            nc.sync.dma_start(
                                out=out_part[qt * 128:(qt + 1) * 128,
                                             dc * 512:(dc + 1) * 512],
                                in_=ot)

    nc.compile()
    return nc


def _host_inputs(x, time_coords, spatial_coords, Wq, bq, Wk, bk, Wv, bv, Wo, bo):
    """Build the 8 per-core input dicts (slicing/layout only)."""
    x = np.asarray(x, np.float32)
    tco = np.asarray(time_coords, np.float32)
    sco = np.asarray(spatial_coords, np.float32)
    mats = {k: np.ascontiguousarray(np.asarray(v, np.float32))
            for k, v in (("Wq", Wq), ("Wk", Wk), ("Wv", Wv), ("Wo", Wo))}
    bqr = np.ascontiguousarray(np.asarray(bq, np.float32).reshape(8, 128).T)
    bkr = np.ascontiguousarray(np.asarray(bk, np.float32).reshape(8, 128).T)
    bvr = np.ascontiguousarray(np.asarray(bv, np.float32).reshape(8, 128).T)
    bo_r = np.asarray(bo, np.float32).reshape(1, D)

    in_maps = []
    for c in range(NCORES):
        b, qs = c // 4, (c % 4) * QS
        t = tco[b]
        a1, a2 = sco[b, :, 0], sco[b, :, 1]
        u = -t * t + a1 * a1 + a2 * a2
        G = np.stack([-u, -np.ones_like(u), -2 * t, 2 * a1, 2 * a2]).astype(np.float32)
        F = np.stack([np.ones_like(u), u, t, a1, a2]).astype(np.float32)
        xTb = np.ascontiguousarray(x[b].T)
        in_maps.append({
            "xT": xTb,
            "xTq": np.ascontiguousarray(xTb[:, qs:qs + QS]),
            "Wq": mats["Wq"], "Wk": mats["Wk"], "Wv": mats["Wv"], "Wo": mats["Wo"],
            "bqr": bqr, "bkr": bkr, "bvr": bvr, "bo": bo_r,
            "G": np.ascontiguousarray(G),
            "Fm": np.ascontiguousarray(F),
            "Gq": np.ascontiguousarray(G[:, qs:qs + QS]),
            "Fq": np.ascontiguousarray(F[:, qs:qs + QS]),
        })
    return in_maps


def kernel(x, time_coords, spatial_coords, Wq, bq, Wk, bk, Wv, bv, Wo, bo,
           _trace=False):
    if "nc" not in _CACHE:
        _CACHE["nc"] = _build_program()
    nc = _CACHE["nc"]
    in_maps = _host_inputs(x, time_coords, spatial_coords,
                           Wq, bq, Wk, bk, Wv, bv, Wo, bo)
    res = bass_utils.run_bass_kernel_spmd(
        nc, in_maps, list(range(NCORES)), trace=_trace)
    _CACHE["last_results"] = res

    out = np.empty((B, T, D), np.float32)
    attn = np.empty((B, H, T, T), np.float32)
    for c in range(NCORES):
        b, qs = c // 4, (c % 4) * QS
        out[b, qs:qs + QS, :] = res.results[c]["out_part"]
        attn[b, :, qs:qs + QS, :] = res.results[c]["attn_part"]
    return out, attn
